# revision 56
# baseline (speedup 1.0000x reference)
"""BitNet transformer block kernel for 8 Trainium2 NeuronCores.

Sharding: data-parallel with K/V dedup. Core c handles batch c//4, token
chunk c%4 (512 tokens). Each core LN+projects K/V only for its own 512
tokens, then the 4 cores of a batch AllGather K+V (one fused 2MB fp16
collective through DRAM). Q projection overlaps the collective.

Attention (no DMA transpose of A): K is stored per-head as [65, 2048]
tiles with a ones row at row 64; Q per-head as [65, 512] with row 64 = 0.
Pass 1 computes q-major logits (contraction 65) solely for the per-query
max (single DVE reduce over [128,2048] PSUM, negated). The negated max is
XBAR-transposed into Q's row 64. Pass 2 recomputes logits K-major --
the matmul itself adds -m via the ones row -- and EXP reads PSUM and
writes transposed A directly in fp16. AV uses V with a ones column per
head so row 64 of the product is the softmax denominator. Head-level
software pipelining interleaves pass1(h) / pass2(h-1) / AV(h-2) /
normalize(h-3) so the PE stays busy (p-state) inside 8 PSUM banks.
"""
import sys

sys.path.insert(0, "/opt/trn_rl_repo")

import numpy as np
from contextlib import ExitStack

import concourse.bass as bass
import concourse.bacc as bacc
import concourse.tile as tile
from concourse import mybir
from concourse.bass_utils import run_bass_kernel_spmd

F32 = mybir.dt.float32
F32R = mybir.dt.float32r
F16 = mybir.dt.float16
AF = mybir.ActivationFunctionType
AX = mybir.AxisListType

DIM = 1024
HEADS = 16
DH = 64
FF = 4096
EPS = 1e-5
T = 2048        # tokens per batch (attention KV scope)
NQ = 512        # own tokens per core
KD = DIM // 128   # 8 feature tiles
N_CORES = 8
NB = T // 128     # 16 token blocks
VW = DH + 1       # V columns per head (64 feats + ones)

_cache = {}


def _quantize(w):
    w = w.astype(np.float32)
    return np.round(np.clip(w, -2.0, 2.0) * np.float32(0.75) + np.float32(0.5)) - np.float32(0.5)


def _prep_weights(i):
    """Host-side: quantize, fold scales/LN-params, transpose to [in, out]."""
    q = {k: _quantize(i[k]) for k in ("wq", "wk", "wv", "wo", "w1", "w2")}
    Wq = q["wq"] * i["sq"][:, None]
    Wk = q["wk"] * i["sk"][:, None]
    Wv = q["wv"] * i["sv"][:, None]
    Wo = q["wo"] * i["so"][:, None]
    W1 = q["w1"] * i["s1"][:, None]
    W2 = q["w2"] * i["s2"][:, None]
    g1, b1n = i["ln1_g"], i["ln1_b"]
    g2, b2n = i["ln2_g"], i["ln2_b"]
    s8 = np.float32(DH ** -0.5)

    def tile_mk(wT):
        """[in, out] -> [128 p, M m-blocks, K k-blocks, 128 mm] so an SBUF
        [128, K, 128] m-block tile loads with one 2KB descriptor per row."""
        ins, outs = wT.shape
        K, M = ins // 128, outs // 128
        return np.ascontiguousarray(
            wT.reshape(K, 128, M, 128).transpose(1, 2, 0, 3).astype(np.float16))

    out = {}
    out["wqT"] = tile_mk((Wq * g1[None, :] * s8).T)
    out["bq"] = ((Wq @ b1n + i["bq"]) * s8).astype(np.float32)
    out["wkT"] = tile_mk((Wk * g1[None, :]).T)
    out["bk"] = (Wk @ b1n + i["bk"]).astype(np.float32)
    # V: [128 p, 2 nb, 8 k, 512 nn]
    wvT = (Wv * g1[None, :]).T
    out["wvT"] = np.ascontiguousarray(
        wvT.reshape(8, 128, 2, 512).transpose(1, 2, 0, 3).astype(np.float16))
    out["woT"] = tile_mk(Wo.T)
    out["bo"] = (Wo @ (Wv @ b1n + i["bv"]) + i["bo"]).astype(np.float32)
    out["w1T"] = tile_mk((W1 * g2[None, :]).T)
    out["b1"] = (W1 @ b2n + i["b1"]).astype(np.float32)
    # W2: [128 p, 4 kh, 8 m, 8 k, 128 mm]
    out["w2T"] = np.ascontiguousarray(
        W2.T.reshape(4, 8, 128, 8, 128).transpose(2, 0, 3, 1, 4).astype(np.float16))
    out["b2"] = i["b2"].astype(np.float32)
    return out


def _ln_chunk(nc, sb, scratch, ps_stat, xh_pool, xt, ones_in, ones_sq, width,
              out_dt=F16):
    """LayerNorm transform of one feature-major chunk [128, KD, width].
    Returns xh = (x - mu) * rstd in out_dt. ones_in must match xt dtype."""
    # 32 output rows (all identical sums): out-partition dims < 32 stream
    # at reduced rate on TRN2
    ssum = ps_stat.tile([32, width], F32, name="ssum")
    ssq = ps_stat.tile([32, width], F32, name="ssq")
    for k in range(KD):
        sq = scratch.tile([128, width], F32R, name="scr", tag="sq")
        nc.scalar.activation(sq[:], xt[:, k], AF.Square)
        nc.tensor.matmul(ssum[:], lhsT=ones_in[:], rhs=xt[:, k],
                         start=(k == 0), stop=(k == KD - 1))
        nc.tensor.matmul(ssq[:], lhsT=ones_sq[:], rhs=sq[:],
                         start=(k == 0), stop=(k == KD - 1))
    mu = sb.tile([1, width], F32R, name="mu")
    nc.vector.tensor_scalar_mul(mu[:], ssum[0:1, :], 1.0 / DIM)
    var = sb.tile([1, width], F32, name="var")
    musq = sb.tile([1, width], F32, name="musq")
    nc.vector.tensor_mul(musq[:], mu[:], mu[:])
    nc.vector.tensor_scalar(var[:], ssq[0:1, :], 1.0 / DIM, None,
                            mybir.AluOpType.mult)
    nc.vector.tensor_sub(var[:], var[:], musq[:])
    nc.vector.tensor_scalar_add(var[:], var[:], float(EPS))
    sd = sb.tile([1, width], F32, name="sd")
    nc.scalar.activation(sd[:], var[:], AF.Sqrt)
    r = sb.tile([1, width], F32R, name="r")
    with nc.allow_low_precision(reason="f32r is fp32 storage"):
        nc.vector.reciprocal(r[:], sd[:])
    mu_b = sb.tile([128, width], F32R, name="mu_b", tag="mu_b")
    r_b = sb.tile([128, width], F32R, name="r_b", tag="r_b")
    nc.gpsimd.partition_broadcast(mu_b[:], mu[:])
    nc.gpsimd.partition_broadcast(r_b[:], r[:])
    xh = []
    for k in range(KD):
        # split normalize across DVE and gpsimd (both SBUF-only here)
        eng = nc.vector if k % 2 == 0 else nc.gpsimd
        xc = scratch.tile([128, width], F32, name="scr2", tag=f"xc{k % 2}")
        eng.tensor_sub(xc[:], xt[:, k], mu_b[:])
        xhk = xh_pool.tile([128, width], out_dt, name=f"xh{k}", tag=f"xh{k}")
        eng.tensor_mul(xhk[:], xc[:], r_b[:])
        xh.append(xhk)
    return xh


NCHUNK = T // NQ  # 4


def _phase_a(nc, tc, d, Ktil, V5, Qtil, bias, ones16, ones32):
    """LN1 + K/V proj over all 4 chunks (redundant per core); Q proj on
    chunk 0 only. Writes the per-head attention layouts directly; the
    odd-head PSUM halves are copied out on DVE to split Act load."""
    xT_t = d["xT"].rearrange("(k p) t -> p k t", p=128)
    with ExitStack() as actx:
        sb_ln = actx.enter_context(tc.tile_pool(name="sb_ln", bufs=1))
        scratch = actx.enter_context(tc.tile_pool(name="scratch", bufs=1))
        sb_xt = actx.enter_context(tc.tile_pool(name="sb_xt", bufs=2))
        sb_xh = actx.enter_context(tc.tile_pool(name="sb_xh", bufs=2))
        wstr = actx.enter_context(tc.tile_pool(name="wstr", bufs=2))
        sb_wv = actx.enter_context(tc.tile_pool(name="sb_wv", bufs=1))
        ps_stat = actx.enter_context(tc.tile_pool(name="ps_stat", bufs=2, space="PSUM"))
        ps_mm = actx.enter_context(tc.tile_pool(name="ps_mm", bufs=4, space="PSUM"))

        def split_copy(kp, even_out, odd_out, bt, m):
            nc.scalar.activation(even_out, kp[0:64, :], AF.Identity,
                                 bias=bias[bt][0:64, m:m + 1])
            with nc.allow_low_precision(reason="fp16 K/Q is the design dtype"):
                nc.vector.tensor_scalar_add(odd_out, kp[64:128, :],
                                            bias[bt][64:128, m:m + 1])

        V5a = V5[:, :, 0:HEADS * VW].rearrange("p b (hh e) -> p b hh e", e=VW)

        def ln_of(c):
            xt = sb_xt.tile([128, KD, NQ], F16, name="xt")
            nc.sync.dma_start(out=xt[:], in_=xT_t[:, :, c * NQ:(c + 1) * NQ])
            return _ln_chunk(nc, sb_ln, scratch, ps_stat, sb_xh, xt,
                             ones16, ones32, NQ)

        xh = ln_of(0)
        for c in range(NCHUNK):
            # K projection (feature-major, split per head with 65-row layout)
            for m in range(KD):
                wk = wstr.tile([128, KD, 128], F16, name="wk", tag="wk")
                nc.sync.dma_start(out=wk[:], in_=d["wkT"][:, m])
                kp = ps_mm.tile([128, NQ], F32, name="kp", tag="mm")
                for k in range(KD):
                    nc.tensor.matmul(kp[:], lhsT=wk[:, k], rhs=xh[k][:],
                                     start=(k == 0), stop=(k == KD - 1))
                cs = slice(c * NQ, (c + 1) * NQ)
                split_copy(kp, Ktil[2 * m][0:64, cs], Ktil[2 * m + 1][0:64, cs],
                           "bk", m)
                if m == 0 and c + 1 < NCHUNK:
                    # emit next chunk's LN now so its stats/normalize overlap
                    # this chunk's projection matmuls
                    xh_next = ln_of(c + 1)
            # V projection (token-major out, per-head 65-wide with ones col)
            for nb in range(2):
                wv = sb_wv.tile([128, KD, NQ], F16, name="wv")
                nc.sync.dma_start(out=wv[:], in_=d["wvT"][:, nb])
                for t_sub in range(4):
                    blk = c * 4 + t_sub
                    vp = ps_mm.tile([128, NQ], F32, name="vp", tag="mm")
                    for k in range(KD):
                        nc.tensor.matmul(
                            vp[:], lhsT=xh[k][:, t_sub * 128:(t_sub + 1) * 128],
                            rhs=wv[:, k], start=(k == 0), stop=(k == KD - 1))
                    vp3 = vp.rearrange("p (hh e) -> p hh e", e=DH)
                    nc.scalar.copy(
                        V5a[:, blk, nb * 8:(nb + 1) * 8, 0:DH], vp3[:])
            if c == 0:
                for m in range(KD):
                    wq = wstr.tile([128, KD, 128], F16, name="wq", tag="wk")
                    nc.sync.dma_start(out=wq[:], in_=d["wqT"][:, m])
                    qp = ps_mm.tile([128, NQ], F32, name="qp", tag="mm")
                    for k in range(KD):
                        nc.tensor.matmul(qp[:], lhsT=wq[:, k], rhs=xh[k][:],
                                         start=(k == 0), stop=(k == KD - 1))
                    split_copy(qp, Qtil[2 * m][0:64, :], Qtil[2 * m + 1][0:64, :],
                               "bq", m)
            if c + 1 < NCHUNK:
                xh = xh_next


def _phase_b(nc, tc, Ktil, V5, Qtil, ATTN_mbs):
    """Attention: two-pass max/exp, transposed A straight from EXP, ones-col
    denominator, 4-deep head pipeline."""
    with ExitStack() as bctx:
        psA = bctx.enter_context(tc.tile_pool(name="psA", bufs=2, space="PSUM"))
        psB = bctx.enter_context(tc.tile_pool(name="psB", bufs=2, space="PSUM"))
        psav = bctx.enter_context(tc.tile_pool(name="psav", bufs=2, space="PSUM"))
        sb_AT = bctx.enter_context(tc.tile_pool(name="sb_AT", bufs=2))
        sb_st = bctx.enter_context(tc.tile_pool(name="sb_st", bufs=2))
        sb_n = bctx.enter_context(tc.tile_pool(name="sb_n", bufs=2))

        AT = {}
        mxh = {}
        avt = {}
        tts = {}

        def norm(h):
            av = avt.pop(h)
            denT = sb_n.tile([1, NQ], F32, name="denT", tag="denT")
            nc.scalar.copy(denT[:], av[64:65, :])
            rd = sb_n.tile([1, NQ], F32, name="rd", tag="rd")
            nc.vector.reciprocal_approx_fast(rd[:], denT[:])
            rdb = sb_n.tile([64, NQ], F32, name="rdb", tag="rdb")
            nc.gpsimd.partition_broadcast(rdb[:], rd[:])
            mb, r0 = h // 2, (h % 2) * 64
            nc.vector.tensor_mul(ATTN_mbs[mb][r0:r0 + 64, :], av[0:64, :],
                                 rdb[:])

        for h in range(HEADS + 3):
            if h >= 3:
                norm(h - 3)
            if h < HEADS:
                mxh[h] = sb_st.tile([128, 128], F16, name="mx", tag="mx")
            for j in range(NB):
                if h < HEADS and j % 2 == 0:
                    qt, half = j // 4, (j // 2) % 2
                    if half == 0:
                        tts[qt] = sb_st.tile([128, 2], F32, name="tt", tag="tt")
                    S = psA.tile([128, T // 2], F32, name="S", tag="S")
                    for cc in range(2):
                        c = half * 2 + cc
                        nc.tensor.matmul(
                            S[:, cc * NQ:(cc + 1) * NQ],
                            lhsT=Qtil[h][:, qt * 128:(qt + 1) * 128],
                            rhs=Ktil[h][:, c * NQ:(c + 1) * NQ],
                            start=True, stop=True)
                    nc.vector.reduce_max(tts[qt][:, half:half + 1], S[:],
                                         axis=AX.X)
                    if half == 1:
                        nc.vector.reduce_max(mxh[h][:, qt:qt + 1],
                                             tts.pop(qt)[:], axis=AX.X,
                                             negate=True)
                if 1 <= h < HEADS + 1:
                    hp = h - 1
                    if j == 0:
                        AT[hp] = sb_AT.tile([128, NB, NQ], F16, name="AT")
                    s2 = psB.tile([128, NQ], F32, name="s2", tag="s2")
                    nc.tensor.matmul(
                        s2[:], lhsT=Ktil[hp][:, j * 128:(j + 1) * 128],
                        rhs=Qtil[hp][:, :], start=True, stop=True)
                    nc.scalar.activation(AT[hp][:, j, :], s2[:], AF.Exp)
                if 2 <= h < HEADS + 2:
                    ha = h - 2
                    if j == 0:
                        avt[ha] = psav.tile([128, NQ], F32, name="av", tag="av")
                    # full 128-col V window: rows 65:128 of the product are
                    # the next head's V x this head's A (garbage, never read)
                    # -- keeps the matmul at full-partition rate
                    nc.tensor.matmul(
                        avt[ha][:, :],
                        lhsT=V5[:, j, ha * VW:ha * VW + 128],
                        rhs=AT[ha][:, j, :],
                        start=(j == 0), stop=(j == NB - 1))
                    if j == NB - 1:
                        AT.pop(ha)
            if h < HEADS:
                # negated maxes -> row 64 of Qtil[h] (XBAR transpose)
                mT = sb_st.tile([128, 128], F16, name="mT", tag="mT")
                nc.sync.dma_start(out=mT[:], in_=mxh.pop(h)[:], transpose=True)
                for qt in range(4):
                    nc.sync.dma_start(
                        out=Qtil[h][64:65, qt * 128:(qt + 1) * 128],
                        in_=mT[qt:qt + 1, 0:128])


def _phase_c(nc, tc, d, ATTN_mbs, bias, ones32):
    """O proj + residual + LN2 + FF + output store."""
    with ExitStack() as cctx:
        sb_ln2 = cctx.enter_context(tc.tile_pool(name="sb_ln2", bufs=2))
        scr2 = cctx.enter_context(tc.tile_pool(name="scr2", bufs=2))
        sb_u = cctx.enter_context(tc.tile_pool(name="sb_u", bufs=1))
        wstr2 = cctx.enter_context(tc.tile_pool(name="wstr2", bufs=4))
        ps_stat2 = cctx.enter_context(tc.tile_pool(name="ps_stat2", bufs=1, space="PSUM"))
        ps_mm2 = cctx.enter_context(tc.tile_pool(name="ps_mm2", bufs=6, space="PSUM"))

        xc0 = sb_u.tile([128, KD, NQ], F16, name="xc0")
        nc.sync.dma_start(
            out=xc0[:],
            in_=d["xT"].rearrange("(k p) t -> p k t", p=128)[:, :, 0:NQ])
        u_sb = sb_u.tile([128, KD, NQ], F32R, name="u_sb")
        for m in range(KD):
            wot = wstr2.tile([128, KD, 128], F16, name="wo", tag="wsm")
            nc.sync.dma_start(out=wot[:], in_=d["woT"][:, m])
            op = ps_mm2.tile([128, NQ], F32, name="op", tag="mm")
            for k in range(KD):
                nc.tensor.matmul(op[:], lhsT=wot[:, k], rhs=ATTN_mbs[k][:],
                                 start=(k == 0), stop=(k == KD - 1))
            upre = scr2.tile([128, NQ], F32, name="upre", tag="scr")
            nc.vector.tensor_add(upre[:], op[:], xc0[:, m])
            nc.scalar.activation(u_sb[:, m], upre[:], AF.Identity,
                                 bias=bias["bo"][:, m:m + 1])
        uh = _ln_chunk(nc, sb_ln2, scr2, ps_stat2, sb_u, u_sb, ones32,
                       ones32, NQ, out_dt=F16)
        H_sb = sb_u.tile([128, FF // 128, NQ], F16, name="H_sb")
        for m in range(FF // 128):
            w1t = wstr2.tile([128, KD, 128], F16, name="w1", tag="wsm")
            nc.sync.dma_start(out=w1t[:], in_=d["w1T"][:, m])
            h1 = ps_mm2.tile([128, NQ], F32, name="h1", tag="mm")
            for k in range(KD):
                nc.tensor.matmul(h1[:], lhsT=w1t[:, k], rhs=uh[k][:],
                                 start=(k == 0), stop=(k == KD - 1))
            nc.scalar.activation(H_sb[:, m], h1[:], AF.Gelu,
                                 bias=bias["b1"][:, m:m + 1])
        for m in range(KD):
            f2 = ps_mm2.tile([128, NQ], F32, name="f2", tag="mm")
            for kh in range(4):
                w2 = wstr2.tile([128, 8, 128], F16, name="w2", tag="w2")
                nc.sync.dma_start(out=w2[:], in_=d["w2T"][:, kh, m])
                for k in range(8):
                    nc.tensor.matmul(f2[:], lhsT=w2[:, k], rhs=H_sb[:, kh * 8 + k],
                                     start=(kh == 0 and k == 0),
                                     stop=(kh == 3 and k == 7))
            opre = scr2.tile([128, NQ], F32, name="opre", tag="scr")
            nc.vector.tensor_add(opre[:], f2[:], u_sb[:, m])
            oout = scr2.tile([128, NQ], F32, name="oout", tag="scr")
            nc.scalar.activation(oout[:], opre[:], AF.Identity,
                                 bias=bias["b2"][:, m:m + 1])
            nc.sync.dma_start(out=d["yT"][m * 128:(m + 1) * 128, :], in_=oout[:])


def _body(nc, tc, d):
    ctx = ExitStack()
    with ctx:
        const = ctx.enter_context(tc.tile_pool(name="const", bufs=1))
        ones_blk = const.tile([128, 128], F32, name="ones_blk")
        nc.vector.memset(ones_blk[:], 1.0)
        ones32 = const.tile([128, 32], F32R, name="ones32")
        nc.vector.tensor_copy(ones32[:], ones_blk[:, 0:32])
        ones16 = const.tile([128, 32], F16, name="ones16")
        nc.vector.tensor_copy(ones16[:], ones_blk[:, 0:32])

        bias = {}
        for nm, n in [("bq", DIM), ("bk", DIM), ("bo", DIM), ("b1", FF), ("b2", DIM)]:
            t = const.tile([128, n // 128], F32, name=f"sb_{nm}")
            nc.sync.dma_start(out=t[:], in_=d[nm].rearrange("(m p) -> p m", p=128))
            bias[nm] = t

        # long-lived activations
        ATTN_mbs = [const.tile([128, NQ], F16, name=f"ATTN_{i}") for i in range(KD)]

        with tc.tile_pool(name="attn_mem", bufs=1) as am:
            Ktil = [am.tile([128, T], F16, name=f"Kt_{h}") for h in range(HEADS)]
            Qtil = [am.tile([128, NQ], F16, name=f"Qt_{h}") for h in range(HEADS)]
            # 63 pad columns so the last head's AV can read a 128-col window
            V5 = am.tile([128, NB, HEADS * VW + 63], F16, name="V5")
            V5o = V5[:, :, 0:HEADS * VW].rearrange("p b (hh e) -> p b hh e", e=VW)
            ones_c = bass.AP(tensor=ones_blk.tensor, offset=ones_blk.offset,
                             ap=[list(ones_blk.ap[0])] + [[0, NB], [0, HEADS], [0, 1]])
            nc.vector.tensor_copy(V5o[:, :, :, DH:DH + 1], ones_c)

            _phase_a(nc, tc, d, Ktil, V5, Qtil, bias, ones16, ones32)
            # ones row (K) / zero pad rows / V pad. Zero-fills go through
            # DMA broadcast (not gpsimd memset: the tile scheduler hoists
            # dep-free gpsimd work to kernel start where it blocks LN's
            # partition broadcasts). Only K needs zero pad rows -- the
            # contraction product is 0 as long as ONE side is zero; Q only
            # needs its -m row zeroed for pass 1.
            with tc.tile_pool(name="init", bufs=1) as ip:
                orow = ip.tile([1, T], F16, name="orow")
                zpad = ip.tile([64, T], F16, name="zpad")
                nc.vector.memset(orow[:], 1.0)
                nc.vector.memset(zpad[:], 0.0)
                nc.gpsimd.memset(V5[:, :, HEADS * VW:], 0.0)
                for h in range(HEADS):
                    nc.sync.dma_start(out=Ktil[h][64:128, :], in_=zpad[:])
                    nc.sync.dma_start(out=Ktil[h][64:65, :], in_=orow[:])
                    # Q pad must be zeroed too: 0 x inf-garbage would be NaN
                    nc.sync.dma_start(out=Qtil[h][64:128, :],
                                      in_=zpad[:, 0:NQ])
            _phase_b(nc, tc, Ktil, V5, Qtil, ATTN_mbs)
        _phase_c(nc, tc, d, ATTN_mbs, bias, ones32)


def _build():
    nc = bacc.Bacc("TRN2", target_bir_lowering=False, debug=False,
                   num_devices=N_CORES)
    d = {}
    d["xT"] = nc.dram_tensor("xT", [DIM, T], F16, kind="ExternalInput").ap()
    d["wqT"] = nc.dram_tensor("wqT", [128, KD, KD, 128], F16, kind="ExternalInput").ap()
    d["wkT"] = nc.dram_tensor("wkT", [128, KD, KD, 128], F16, kind="ExternalInput").ap()
    d["wvT"] = nc.dram_tensor("wvT", [128, 2, KD, NQ], F16, kind="ExternalInput").ap()
    d["woT"] = nc.dram_tensor("woT", [128, KD, KD, 128], F16, kind="ExternalInput").ap()
    d["w1T"] = nc.dram_tensor("w1T", [128, FF // 128, KD, 128], F16, kind="ExternalInput").ap()
    d["w2T"] = nc.dram_tensor("w2T", [128, 4, KD, 8, 128], F16, kind="ExternalInput").ap()
    for nm, n in [("bq", DIM), ("bk", DIM), ("bo", DIM), ("b1", FF), ("b2", DIM)]:
        d[nm] = nc.dram_tensor(nm, [n], F32, kind="ExternalInput").ap()
    d["yT"] = nc.dram_tensor("yT", [DIM, NQ], F32, kind="ExternalOutput").ap()
    with tile.TileContext(nc) as tc:
        _body(nc, tc, d)
    nc.compile()
    return nc


def _in_maps(inputs):
    x = inputs["x"].astype(np.float32)
    B = x.shape[0]
    w = _prep_weights(inputs)
    per_batch = N_CORES // B
    maps = []
    for c in range(N_CORES):
        b, chunk = divmod(c, per_batch)
        xT = np.ascontiguousarray(
            np.roll(x[b].T, -chunk * NQ, axis=1)).astype(np.float16)
        m = {"xT": xT}
        m.update(w)
        maps.append(m)
    return maps


def kernel(**inputs) -> np.ndarray:
    inputs = {k: np.asarray(v) for k, v in inputs.items()}
    x = inputs["x"].astype(np.float32)
    B, N, D = x.shape  # (2, 2048, 1024)

    if "nc" not in _cache:
        _cache["nc"] = _build()
    nc = _cache["nc"]

    res = run_bass_kernel_spmd(nc, _in_maps(inputs), core_ids=list(range(N_CORES)))
    per_batch = N_CORES // B
    out = np.empty((B, N, D), dtype=np.float32)
    for c in range(N_CORES):
        b, chunk = divmod(c, per_batch)
        out[b, chunk * NQ:(chunk + 1) * NQ, :] = res.results[c]["yT"].T
    return out
